# revision 3
# baseline (speedup 1.0000x reference)
"""DotProductDistributionHead — bf16 SWDGE gather with sorted-rank adaptive
grouping (zero padding).

Instead of 4 fixed vocab passes (whose per-partition count variance forces
~36% rectangle padding), each partition's 200 candidates are SORTED and
grouped by rank: group g covers ranks [r_g, r_{g+1}) so every partition
contributes exactly the same column count — zero padding. The int16 index
constraint is met by choosing group boundaries greedily on the host: a group
is valid iff (max - min) of its member indices across all cores/partitions
fits in 32767; the per-group min is baked into the gather's source offset.
Random-uniform candidates yield ~7 groups per block (~27 gathers/core,
102,400 descriptors vs 139,584 for the pass scheme).

Everything else matches the pass-based kernel: bf16 table + h, partition-
aligned DVE broadcast-multiply + segmented reduce, host reassembly via the
sort permutation.
"""

import os
import sys
import types

import numpy as np
import ml_dtypes

import concourse.bacc as bacc
import concourse.tile as tile
from concourse import mybir
from concourse.bass_utils import run_bass_kernel_spmd

B, N, D, V = 4096, 200, 128, 100000
NCORES = 8
B_LOC = B // NCORES          # 512 batch rows per core
NBLK = B_LOC // 128          # 4 blocks of 128 rows
IDX_RANGE = 32767            # int16 gather index limit

CHUNK_T = int(os.environ.get("KERNEL_CHUNK_T", 24))
GBUFS = int(os.environ.get("KERNEL_GBUFS", 4))

TRACE = False
LAST_RESULTS = None
ACT_FUNC = "Gelu"

_f32 = mybir.dt.float32
_bf16 = mybir.dt.bfloat16
_i16 = mybir.dt.int16

CONST_COLS = D + B_LOC + D   # [W | xT | b_mu replicated]

_program_cache = {}


def _install_ntff_shim():
    try:
        try:
            import antenv.axon_hooks  # noqa: F401
            return
        except ImportError:
            pass
        import antenv
        mod = types.ModuleType("antenv.axon_hooks")
        _store = {}
        mod.set_axon_ntff_profile_hook = lambda h: _store.__setitem__("h", h)
        mod.get_axon_ntff_profile_hook = lambda: _store.get("h")
        sys.modules["antenv.axon_hooks"] = mod
        antenv.axon_hooks = mod
        from trn_agent_boot.trn_boot import _ntff_profile_via_ctypes
        hook = _ntff_profile_via_ctypes("/opt/axon/libaxon_pjrt.so")
        if hook is not None:
            mod.set_axon_ntff_profile_hook(hook)
    except Exception:
        pass


_install_ntff_shim()


def _chunks(total):
    out = []
    t0 = 0
    while t0 < total:
        out.append((t0, min(CHUNK_T, total - t0)))
        t0 += CHUNK_T
    return out


def _kernel_body(tc, consts, gidx, emb, out, groups):
    """groups: per block, list of (base_row, T)."""
    nc = tc.nc
    gelu = getattr(mybir.ActivationFunctionType, ACT_FUNC)
    total_words = sum(t * 8 for blk in groups for _, t in blk)

    with (
        tc.tile_pool(name="const", bufs=1) as cpool,
        tc.tile_pool(name="psum", bufs=2, space="PSUM") as ppool,
        tc.tile_pool(name="outs", bufs=2) as outpool,
        tc.tile_pool(name="gather", bufs=GBUFS) as gpool,
        tc.tile_pool(name="scratch", bufs=2) as spool,
    ):
        c_sb = cpool.tile([128, CONST_COLS], _f32)
        nc.sync.dma_start(c_sb[:], consts[:, :])
        W_sb = c_sb[:, 0:D]
        xT_sb = c_sb[:, D : D + B_LOC]
        bias_sb = c_sb[:, D + B_LOC : D + B_LOC + D]

        # split the index load so the first gather isn't gated on the full tile
        w0 = groups[0][0][1] * 8
        gidx_sb = cpool.tile([128, total_words], _i16)
        nc.sync.dma_start(gidx_sb[:, :w0], gidx[:, :w0])
        if total_words > w0:
            nc.sync.dma_start(gidx_sb[:, w0:], gidx[:, w0:])

        h_sb = cpool.tile([128, NBLK * D], _bf16)
        for c in range(NBLK):
            ps = ppool.tile([128, D], _f32)
            nc.tensor.matmul(
                out=ps[:], lhsT=xT_sb[:, c * 128 : (c + 1) * 128], rhs=W_sb,
                start=True, stop=True,
            )
            nc.vector.tensor_tensor(
                out=ps[:], in0=ps[:], in1=bias_sb, op=mybir.AluOpType.add
            )
            nc.scalar.activation(out=h_sb[:, c * D : (c + 1) * D], in_=ps[:], func=gelu)

        qrr = 0
        word_off = 0
        for c in range(NBLK):
            h_blk = h_sb[:, c * D : (c + 1) * D]
            logits_sb = outpool.tile([128, N], _f32)
            col = 0
            for base, T in groups[c]:
                emb_k = emb[base:, :]
                for t0, tc_len in _chunks(T):
                    num = 128 * tc_len
                    G = gpool.tile([128, CHUNK_T * D], _bf16, tag="gtile")
                    G3 = G[:, : tc_len * D].rearrange("p (t d) -> p t d", d=D)
                    nc.gpsimd.dma_gather(
                        out_ap=G3,
                        in_ap=emb_k,
                        idxs_ap=gidx_sb[:, word_off + t0 * 8 : word_off + (t0 + tc_len) * 8],
                        num_idxs=num,
                        num_idxs_reg=num,
                        elem_size=D,
                        single_packet=False,
                        queue_num=qrr % 4,
                    )
                    qrr += 1
                    h_bc = h_blk.unsqueeze(1).to_broadcast([128, tc_len, D])
                    prod = spool.tile([128, CHUNK_T * D], _bf16, tag="ptile")
                    p3 = prod[:, : tc_len * D].rearrange("p (t d) -> p t d", d=D)
                    nc.vector.tensor_tensor(out=p3, in0=G3, in1=h_bc,
                                            op=mybir.AluOpType.mult)
                    nc.vector.tensor_reduce(
                        out=logits_sb[:, col + t0 : col + t0 + tc_len], in_=p3,
                        axis=mybir.AxisListType.X, op=mybir.AluOpType.add,
                    )
                word_off += T * 8
                col += T
            nc.sync.dma_start(out[:, c * N : (c + 1) * N], logits_sb[:])


def _build_program(groups):
    key = tuple(tuple(blk) for blk in groups) + (ACT_FUNC, CHUNK_T, GBUFS)
    if key in _program_cache:
        return _program_cache[key]
    nc = bacc.Bacc(
        "TRN2",
        target_bir_lowering=False,
        debug=False,
        enable_asserts=False,
        num_devices=NCORES,
        num_swdge_queues=4,
    )
    total_words = sum(t * 8 for blk in groups for _, t in blk)
    consts = nc.dram_tensor("consts", (128, CONST_COLS), _f32, kind="ExternalInput").ap()
    gidx = nc.dram_tensor("gidx", (128, total_words), _i16, kind="ExternalInput").ap()
    emb = nc.dram_tensor("emb", (V, D), _bf16, kind="ExternalInput").ap()
    out = nc.dram_tensor("out", (128, NBLK * N), _f32, kind="ExternalOutput").ap()
    with tile.TileContext(nc) as tc:
        _kernel_body(tc, consts, gidx, emb, out, groups)
    nc.finalize()
    _program_cache[key] = nc
    return nc


def prepare(x, candidates, W_mu, b_mu, mu_bias, emb_table):
    x = np.asarray(x, dtype=np.float32)
    candidates = np.asarray(candidates).astype(np.int64)
    W_mu = np.ascontiguousarray(np.asarray(W_mu, dtype=np.float32))
    b_mu = np.asarray(b_mu, dtype=np.float32)
    emb = np.ascontiguousarray(np.asarray(emb_table).astype(ml_dtypes.bfloat16))

    # core, block, partition, n
    cand4 = candidates.reshape(NCORES, NBLK, 128, N)
    order = np.argsort(cand4, axis=3, kind="stable")
    sorted_c = np.take_along_axis(cand4, order, axis=3)

    # per block: greedy rank-group boundaries valid across all cores/partitions
    groups = []
    for c in range(NBLK):
        s = sorted_c[:, c].reshape(-1, N)       # [1024, 200]
        cmin = s.min(axis=0)
        cmax = s.max(axis=0)
        blk_groups = []
        r = 0
        while r < N:
            assert cmax[r] - cmin[r] <= IDX_RANGE, "single rank exceeds int16 range"
            hi = r + 1
            while hi < N and cmax[hi] - cmin[r] <= IDX_RANGE:
                hi += 1
            blk_groups.append((int(cmin[r]), int(hi - r)))
            r = hi
        groups.append(blk_groups)

    nc = _build_program(groups)

    total_words = sum(t * 8 for blk in groups for _, t in blk)

    gidx_tiles = []
    col_maps = []  # per core: [B_LOC, N] -> column in out
    inv = np.empty_like(order)
    np.put_along_axis(inv, order, np.broadcast_to(np.arange(N), order.shape), axis=3)
    for core in range(NCORES):
        words = np.zeros((128, total_words), dtype=np.uint16)
        woff = 0
        for c in range(NBLK):
            r = 0
            for base, T in groups[c]:
                I = (sorted_c[core, c, :, r : r + T] - base).astype(np.uint16)
                # wrap: value for (p_dest=j%128, t=j//128) at word (j%16, j//16)
                vals_flat = I.T.ravel()  # j = t*128 + p
                wrapped = vals_flat.reshape(T * 8, 16).T  # [16, T*8]
                words[:, woff : woff + T * 8] = np.tile(wrapped, (8, 1))
                woff += T * 8
                r += T
        gidx_tiles.append(words.view(np.int16))
        # rank r of (block c, partition p) lands at out column c*N + r
        cm = (inv[core] + (np.arange(NBLK) * N)[:, None, None]).reshape(B_LOC, N)
        col_maps.append(cm)

    bias_tile = np.broadcast_to(b_mu.reshape(1, D), (128, D))
    in_maps = []
    for core in range(NCORES):
        sl = slice(core * B_LOC, (core + 1) * B_LOC)
        consts = np.concatenate([W_mu, x[sl].T, bias_tile], axis=1)
        in_maps.append(
            {
                "consts": np.ascontiguousarray(consts, dtype=np.float32),
                "gidx": np.ascontiguousarray(gidx_tiles[core]),
                "emb": emb,
            }
        )
    return nc, in_maps, col_maps


def assemble(results, col_maps):
    logits = np.zeros((B, N), dtype=np.float32)
    for core in range(len(results)):
        out_core = results[core]["out"]
        cm = col_maps[core]
        for c in range(NBLK):
            rows = slice(c * 128, (c + 1) * 128)
            logits[core * B_LOC + c * 128 : core * B_LOC + (c + 1) * 128] = (
                np.take_along_axis(out_core, cm[rows], axis=1)
            )
    return logits


def kernel(x, candidates, W_mu, b_mu, mu_bias, emb_table):
    global LAST_RESULTS
    candidates = np.asarray(candidates).astype(np.int64)
    mu_bias = np.asarray(mu_bias, dtype=np.float32)
    nc, in_maps, col_maps = prepare(x, candidates, W_mu, b_mu, mu_bias, emb_table)
    ncores_run = int(os.environ.get("KERNEL_CORES", NCORES))
    res = run_bass_kernel_spmd(
        nc, in_maps[:ncores_run], core_ids=list(range(ncores_run)), trace=TRACE
    )
    LAST_RESULTS = res
    logits = assemble(res.results[:ncores_run], col_maps[:ncores_run])
    if np.any(mu_bias):
        logits = logits + mu_bias[candidates]
    return np.ascontiguousarray(logits.astype(np.float32))


# revision 4
# speedup vs baseline: 1.3114x; 1.3114x over previous
"""DotProductDistributionHead — bf16 SWDGE gather with sorted-rank adaptive
grouping (zero padding).

Instead of 4 fixed vocab passes (whose per-partition count variance forces
~36% rectangle padding), each partition's 200 candidates are SORTED and
grouped by rank: group g covers ranks [r_g, r_{g+1}) so every partition
contributes exactly the same column count — zero padding. The int16 index
constraint is met by choosing group boundaries greedily on the host: a group
is valid iff (max - min) of its member indices across all cores/partitions
fits in 32767; the per-group min is baked into the gather's source offset.
Random-uniform candidates yield ~7 groups per block (~27 gathers/core,
102,400 descriptors vs 139,584 for the pass scheme).

Everything else matches the pass-based kernel: bf16 table + h, partition-
aligned DVE broadcast-multiply + segmented reduce, host reassembly via the
sort permutation.
"""

import os
import sys
import types

import numpy as np
import ml_dtypes

import concourse.bacc as bacc
import concourse.tile as tile
from concourse import mybir
from concourse.bass_utils import run_bass_kernel_spmd

B, N, D, V = 4096, 200, 128, 100000
NCORES = 8
B_LOC = B // NCORES          # 512 batch rows per core
NBLK = B_LOC // 128          # 4 blocks of 128 rows
IDX_RANGE = 32767            # int16 gather index limit

CHUNK_T = int(os.environ.get("KERNEL_CHUNK_T", 24))
GBUFS = int(os.environ.get("KERNEL_GBUFS", 8))

TRACE = False
LAST_RESULTS = None
ACT_FUNC = "Gelu"

_f32 = mybir.dt.float32
_bf16 = mybir.dt.bfloat16
_i16 = mybir.dt.int16

CONST_COLS = D + B_LOC + D   # [W | xT | b_mu replicated]

_program_cache = {}


def _install_ntff_shim():
    try:
        try:
            import antenv.axon_hooks  # noqa: F401
            return
        except ImportError:
            pass
        import antenv
        mod = types.ModuleType("antenv.axon_hooks")
        _store = {}
        mod.set_axon_ntff_profile_hook = lambda h: _store.__setitem__("h", h)
        mod.get_axon_ntff_profile_hook = lambda: _store.get("h")
        sys.modules["antenv.axon_hooks"] = mod
        antenv.axon_hooks = mod
        from trn_agent_boot.trn_boot import _ntff_profile_via_ctypes
        hook = _ntff_profile_via_ctypes("/opt/axon/libaxon_pjrt.so")
        if hook is not None:
            mod.set_axon_ntff_profile_hook(hook)
    except Exception:
        pass


_install_ntff_shim()


def _chunks(total):
    out = []
    t0 = 0
    while t0 < total:
        out.append((t0, min(CHUNK_T, total - t0)))
        t0 += CHUNK_T
    return out


def _kernel_body(tc, consts, gidx, emb, out, groups):
    """groups: per block, list of (base_row, T)."""
    nc = tc.nc
    gelu = getattr(mybir.ActivationFunctionType, ACT_FUNC)
    total_words = sum(t * 8 for blk in groups for _, t in blk)

    with (
        tc.tile_pool(name="const", bufs=1) as cpool,
        tc.tile_pool(name="psum", bufs=2, space="PSUM") as ppool,
        tc.tile_pool(name="outs", bufs=2) as outpool,
        tc.tile_pool(name="gather", bufs=GBUFS) as gpool,
        tc.tile_pool(name="scratch", bufs=2) as spool,
    ):
        c_sb = cpool.tile([128, CONST_COLS], _f32)
        nc.sync.dma_start(c_sb[:], consts[:, :])
        W_sb = c_sb[:, 0:D]
        xT_sb = c_sb[:, D : D + B_LOC]
        bias_sb = c_sb[:, D + B_LOC : D + B_LOC + D]

        # split the index load so the first gather isn't gated on the full tile
        w0 = groups[0][0][1] * 8
        gidx_sb = cpool.tile([128, total_words], _i16)
        nc.sync.dma_start(gidx_sb[:, :w0], gidx[:, :w0])
        if total_words > w0:
            nc.sync.dma_start(gidx_sb[:, w0:], gidx[:, w0:])

        h_sb = cpool.tile([128, NBLK * D], _bf16)
        for c in range(NBLK):
            ps = ppool.tile([128, D], _f32)
            nc.tensor.matmul(
                out=ps[:], lhsT=xT_sb[:, c * 128 : (c + 1) * 128], rhs=W_sb,
                start=True, stop=True,
            )
            nc.vector.tensor_tensor(
                out=ps[:], in0=ps[:], in1=bias_sb, op=mybir.AluOpType.add
            )
            nc.scalar.activation(out=h_sb[:, c * D : (c + 1) * D], in_=ps[:], func=gelu)

        qrr = 0
        word_off = 0
        for c in range(NBLK):
            h_blk = h_sb[:, c * D : (c + 1) * D]
            logits_sb = outpool.tile([128, N], _f32)
            col = 0
            for base, T in groups[c]:
                emb_k = emb[base:, :]
                for t0, tc_len in _chunks(T):
                    num = 128 * tc_len
                    G = gpool.tile([128, CHUNK_T * D], _bf16, tag="gtile")
                    G3 = G[:, : tc_len * D].rearrange("p (t d) -> p t d", d=D)
                    nc.gpsimd.dma_gather(
                        out_ap=G3,
                        in_ap=emb_k,
                        idxs_ap=gidx_sb[:, word_off + t0 * 8 : word_off + (t0 + tc_len) * 8],
                        num_idxs=num,
                        num_idxs_reg=num,
                        elem_size=D,
                        single_packet=False,
                        queue_num=qrr % 4,
                    )
                    qrr += 1
                    h_bc = h_blk.unsqueeze(1).to_broadcast([128, tc_len, D])
                    prod = spool.tile([128, CHUNK_T * D], _bf16, tag="ptile")
                    p3 = prod[:, : tc_len * D].rearrange("p (t d) -> p t d", d=D)
                    nc.vector.tensor_tensor(out=p3, in0=G3, in1=h_bc,
                                            op=mybir.AluOpType.mult)
                    nc.vector.tensor_reduce(
                        out=logits_sb[:, col + t0 : col + t0 + tc_len], in_=p3,
                        axis=mybir.AxisListType.X, op=mybir.AluOpType.add,
                    )
                word_off += T * 8
                col += T
            nc.sync.dma_start(out[:, c * N : (c + 1) * N], logits_sb[:])


def _build_program(groups):
    key = tuple(tuple(blk) for blk in groups) + (ACT_FUNC, CHUNK_T, GBUFS)
    if key in _program_cache:
        return _program_cache[key]
    nc = bacc.Bacc(
        "TRN2",
        target_bir_lowering=False,
        debug=False,
        enable_asserts=False,
        num_devices=NCORES,
        num_swdge_queues=4,
    )
    total_words = sum(t * 8 for blk in groups for _, t in blk)
    consts = nc.dram_tensor("consts", (128, CONST_COLS), _f32, kind="ExternalInput").ap()
    gidx = nc.dram_tensor("gidx", (128, total_words), _i16, kind="ExternalInput").ap()
    emb = nc.dram_tensor("emb", (V, D), _bf16, kind="ExternalInput").ap()
    out = nc.dram_tensor("out", (128, NBLK * N), _f32, kind="ExternalOutput").ap()
    with tile.TileContext(nc) as tc:
        _kernel_body(tc, consts, gidx, emb, out, groups)
    nc.finalize()
    _program_cache[key] = nc
    return nc


def prepare(x, candidates, W_mu, b_mu, mu_bias, emb_table):
    x = np.asarray(x, dtype=np.float32)
    candidates = np.asarray(candidates).astype(np.int64)
    W_mu = np.ascontiguousarray(np.asarray(W_mu, dtype=np.float32))
    b_mu = np.asarray(b_mu, dtype=np.float32)
    emb = np.ascontiguousarray(np.asarray(emb_table).astype(ml_dtypes.bfloat16))

    # core, block, partition, n
    cand4 = candidates.reshape(NCORES, NBLK, 128, N)
    order = np.argsort(cand4, axis=3, kind="stable")
    sorted_c = np.take_along_axis(cand4, order, axis=3)

    # per block: greedy rank-group boundaries valid across all cores/partitions
    groups = []
    for c in range(NBLK):
        s = sorted_c[:, c].reshape(-1, N)       # [1024, 200]
        cmin = s.min(axis=0)
        cmax = s.max(axis=0)
        blk_groups = []
        r = 0
        while r < N:
            assert cmax[r] - cmin[r] <= IDX_RANGE, "single rank exceeds int16 range"
            hi = r + 1
            while hi < N and cmax[hi] - cmin[r] <= IDX_RANGE:
                hi += 1
            blk_groups.append((int(cmin[r]), int(hi - r)))
            r = hi
        groups.append(blk_groups)

    nc = _build_program(groups)

    total_words = sum(t * 8 for blk in groups for _, t in blk)

    gidx_tiles = []
    col_maps = []  # per core: [B_LOC, N] -> column in out
    inv = np.empty_like(order)
    np.put_along_axis(inv, order, np.broadcast_to(np.arange(N), order.shape), axis=3)
    for core in range(NCORES):
        words = np.zeros((128, total_words), dtype=np.uint16)
        woff = 0
        for c in range(NBLK):
            r = 0
            for base, T in groups[c]:
                I = (sorted_c[core, c, :, r : r + T] - base).astype(np.uint16)
                # wrap: value for (p_dest=j%128, t=j//128) at word (j%16, j//16)
                vals_flat = I.T.ravel()  # j = t*128 + p
                wrapped = vals_flat.reshape(T * 8, 16).T  # [16, T*8]
                words[:, woff : woff + T * 8] = np.tile(wrapped, (8, 1))
                woff += T * 8
                r += T
        gidx_tiles.append(words.view(np.int16))
        # rank r of (block c, partition p) lands at out column c*N + r
        cm = (inv[core] + (np.arange(NBLK) * N)[:, None, None]).reshape(B_LOC, N)
        col_maps.append(cm)

    bias_tile = np.broadcast_to(b_mu.reshape(1, D), (128, D))
    in_maps = []
    for core in range(NCORES):
        sl = slice(core * B_LOC, (core + 1) * B_LOC)
        consts = np.concatenate([W_mu, x[sl].T, bias_tile], axis=1)
        in_maps.append(
            {
                "consts": np.ascontiguousarray(consts, dtype=np.float32),
                "gidx": np.ascontiguousarray(gidx_tiles[core]),
                "emb": emb,
            }
        )
    return nc, in_maps, col_maps


def assemble(results, col_maps):
    logits = np.zeros((B, N), dtype=np.float32)
    for core in range(len(results)):
        out_core = results[core]["out"]
        cm = col_maps[core]
        for c in range(NBLK):
            rows = slice(c * 128, (c + 1) * 128)
            logits[core * B_LOC + c * 128 : core * B_LOC + (c + 1) * 128] = (
                np.take_along_axis(out_core, cm[rows], axis=1)
            )
    return logits


def kernel(x, candidates, W_mu, b_mu, mu_bias, emb_table):
    global LAST_RESULTS
    candidates = np.asarray(candidates).astype(np.int64)
    mu_bias = np.asarray(mu_bias, dtype=np.float32)
    nc, in_maps, col_maps = prepare(x, candidates, W_mu, b_mu, mu_bias, emb_table)
    ncores_run = int(os.environ.get("KERNEL_CORES", NCORES))
    res = run_bass_kernel_spmd(
        nc, in_maps[:ncores_run], core_ids=list(range(ncores_run)), trace=TRACE
    )
    LAST_RESULTS = res
    logits = assemble(res.results[:ncores_run], col_maps[:ncores_run])
    if np.any(mu_bias):
        logits = logits + mu_bias[candidates]
    return np.ascontiguousarray(logits.astype(np.float32))
